# revision 2
# baseline (speedup 1.0000x reference)
"""MoE router kernel for Trainium2 (8 NeuronCores, SPMD data-parallel), v3.

Computes, for x [B,S,H] and gate_w [E,H]:
    logits = x @ gate_w.T; p = softmax(logits); top-2 renormalized
    => w1 = sigmoid(l1 - l2), w2 = sigmoid(l2 - l1).

v3 = hybrid input engine:
- Host splits x into bf16 hi/lo (same total bytes as fp32) and gate_w
  into stacked bf16 blocks Wa = [w_hi | w_lo], Wb = [w_lo | w_hi].
- Groups 0-2 (6 slabs) load via DMA xbar transpose (HBM -> SBUF already
  transposed), all serialized on the sync queue (the xbar is a single
  shared unit; concurrent xbar transposes corrupt; plain DMA alongside
  is safe - probed).
- Group 3 loads via plain DMA (scalar queue) + PE bf16 transposes,
  statically interleaved into PE idle slots so the PE stays warm.
- GEMM per h-chunk c accumulates into PSUM bank A [128, 512]:
    MM_a: lhsT = [w_hi_c | w_lo_c], rhs = hiT_c
    MM_b: lhsT = [w_lo_c | w_hi_c], rhs = loT_c
  logits = A[0:64] + A[64:128] = (hi+lo) @ (w_hi+w_lo)  (~17-bit).
  The halves-add is folded into the transpose-back matmul vs [I64; I64].
- Output DMAs are deferred to the end (they must not block the xbar
  stream mid-kernel).
"""

import sys

sys.path.insert(0, "/opt/trn_rl_repo")

import numpy as np

import concourse.bass as bass
import concourse.mybir as mybir
import concourse.tile as tile
from concourse.bass_utils import run_bass_kernel_spmd
import orjson
import concourse.bass_utils as _bu
import concourse.bass2jax as _b2j

_orig_compile_bir = _bu.compile_bir_kernel


def _legalize_waits(bir_json: bytes) -> bytes:
    """This walrus build allows only ONE sync-wait per compute
    instruction; move excess waits onto a Drain inserted just before."""
    m = orjson.loads(bir_json)
    changed = False
    for fn in m["functions"]:
        for blk in fn["blocks"]:
            out = []
            for inst in blk["instructions"]:
                si = inst.get("sync_info")
                w = (si or {}).get("on_wait") or []
                if len(w) > 1:
                    for k, wk in enumerate(w[:-1]):
                        out.append({
                            "debug": inst.get("debug", 0),
                            "engine": inst["engine"],
                            "ins": [], "outs": [],
                            "name": inst["name"] + f"-lw{k}",
                            "opcode": "Drain",
                            "sync_info": {"on_update": [], "on_wait": [wk]},
                        })
                    si["on_wait"] = w[-1:]
                    changed = True
                out.append(inst)
            blk["instructions"] = out
    return orjson.dumps(m) if changed else bir_json


def _compile_bir_legalized(bir_json, tmpdir, neff_name="file.neff"):
    return _orig_compile_bir(_legalize_waits(bir_json), tmpdir, neff_name)


_bu.compile_bir_kernel = _compile_bir_legalized
_b2j.compile_bir_kernel = _compile_bir_legalized

F32 = mybir.dt.float32
U32 = mybir.dt.uint32
BF16 = mybir.dt.bfloat16

B, S, H, E = 4, 4096, 4096, 64
N_CORES = 8
P = 128
TOK_TOTAL = B * S            # 16384
TOK = TOK_TOTAL // N_CORES   # 2048 tokens per core
NCH = H // P                 # 32 contraction chunks of 128
GTOK = 512                   # tokens per group (one PSUM bank of fp32)
NG = TOK // GTOK             # 4 groups
PE_GROUP = 3                 # this group's slabs go via plain DMA + PE


def build_nc(tok: int = TOK):
    nc = bass.Bass()

    xhi_ext = nc.declare_dram_parameter("xhi", [tok, H], BF16, isOutput=False)
    xlo_ext = nc.declare_dram_parameter("xlo", [tok, H], BF16, isOutput=False)
    wa_ext = nc.declare_dram_parameter("wa", [P, NCH, P], BF16, isOutput=False)
    sid_ext = nc.declare_dram_parameter("sumid", [P, E], F32, isOutput=False)
    ow_ext = nc.declare_dram_parameter("out_w", [tok, 2], F32, isOutput=True)
    oi_ext = nc.declare_dram_parameter("out_i", [tok, 2], U32, isOutput=True)

    with tile.TileContext(nc) as tc:
        with (
            tc.tile_pool(name="consts", bufs=1) as consts,
            tc.tile_pool(name="xb", bufs=5) as xbp,        # xbar-landed slabs
            tc.tile_pool(name="ps_a", bufs=2, space="PSUM") as psA,
            tc.tile_pool(name="ps_t", bufs=2, space="PSUM") as psT,
            tc.tile_pool(name="lgp", bufs=2) as lgp,
            tc.tile_pool(name="small", bufs=4) as small,
            tc.tile_pool(name="outp", bufs=2 * NG) as outp,
        ):
            # ---- consts (scalar queue; sync queue is reserved for xbar) ----
            wa_sb = consts.tile([P, NCH, P], BF16)
            nc.sync.dma_start(wa_sb[:], wa_ext[:])
            sid_sb = consts.tile([P, E], F32)
            nc.sync.dma_start(sid_sb[:], sid_ext[:])

            # Engine primers.
            prim = consts.tile([P, 2], F32)
            nc.vector.memset(prim[:, 0:1], 0.0)
            nc.scalar.copy(prim[:, 1:2], nc.const_aps.tensor(1.0, (P, 1)))
            with tc.tile_pool(name="scr", bufs=1, space="PSUM") as scr_pool:
                scr = scr_pool.tile([P, P], F32)
                nc.tensor.matmul(scr[0:E, 0:E], sid_sb[:], sid_sb[:],
                                 start=True, stop=True)
                nc.tensor.matmul(scr[:], wa_sb[:, 0, :], wa_sb[:, 0, :],
                                 start=True, stop=True)

            # ---- xbar transposes for groups 0-2 (sync queue, serial) ----
            slabs = {}
            for half, g in (("hi", 0), ("lo", 0), ("hi", 1), ("lo", 1),
                            ("hi", 2), ("lo", 2), ("hi", 3), ("lo", 3)):
                slabs[(half, g)] = xbp.tile([P, NCH, GTOK], BF16,
                                            name="xslab")

            def xbar(half, g, c0=0, c1=NCH):
                t0 = g * GTOK
                ext = xhi_ext if half == "hi" else xlo_ext
                nc.sync.dma_start(
                    slabs[(half, g)][:, c0:c1],
                    ext[t0:t0 + GTOK, c0 * P:c1 * P],
                    transpose=True)

            xbar("hi", 0)
            xbar("lo", 0)
            xbar("hi", 1)
            xbar("lo", 1)

            # ---- PE-side helpers ----
            A_tiles = {}

            def gemm_half(g, half, c0=0, c1=NCH, first=False, last=False):
                A = A_tiles[g]
                w_sb = wa_sb
                src = slabs[(half, g)]
                for c in range(c0, c1):
                    nc.tensor.matmul(A[:], w_sb[:, c, :], src[:, c, :],
                                     start=(first and c == c0),
                                     stop=(last and c == c1 - 1))

            def finish_group(g):
                """As copy (DVE), sum-matmuls, lgt copy, topk, sigmoid."""
                A = A_tiles[g]
                As = lgp.tile([P, GTOK], F32)
                nc.vector.tensor_copy(As[:], A[:])
                ltb = psT.tile([P, 4, E], F32)
                for t in range(4):
                    nc.tensor.matmul(ltb[:, t, :],
                                     As[:, t * P:(t + 1) * P], sid_sb[:],
                                     start=True, stop=True)
                lgt = lgp.tile([P, 4, E], F32)
                nc.scalar.copy(lgt[:], ltb[:])
                ow_t = outp.tile([P, 4, 2], F32)
                oi_t = outp.tile([P, 4, 2], U32)
                for t in range(4):
                    mx = small.tile([P, 8], F32)
                    nc.vector.max(mx[:], lgt[:, t, :])
                    ix = small.tile([P, 8], U32)
                    nc.vector.max_index(ix[:], mx[:], lgt[:, t, :])
                    nc.scalar.activation(
                        ow_t[:, t, 0:1], mx[:, 1:2],
                        mybir.ActivationFunctionType.Sigmoid,
                        bias=mx[:, 0:1], scale=-1.0)
                    nc.scalar.activation(
                        ow_t[:, t, 1:2], mx[:, 0:1],
                        mybir.ActivationFunctionType.Sigmoid,
                        bias=mx[:, 1:2], scale=-1.0)
                    nc.vector.tensor_copy(oi_t[:, t, :], ix[:, 0:2])
                return ow_t, oi_t

            # ---- static PE schedule (interleave transposes into idle) ----
            for g in (0, 3, 1, 2):
                A_tiles[g] = psA.tile([P, GTOK], F32, name="A")

            def emit_out(g, ow_t, oi_t):
                t0 = g * GTOK
                nc.sync.dma_start(
                    ow_ext[t0:t0 + GTOK, :].rearrange("(t p) k -> p t k", p=P),
                    ow_t[:])
                nc.sync.dma_start(
                    oi_ext[t0:t0 + GTOK, :].rearrange("(t p) k -> p t k", p=P),
                    oi_t[:])

            # Interleave each group's output DMAs into the sync queue two
            # slab-pairs later: they wait on the group's compute, pacing
            # the xbar stream so slab-buffer reuse can never outrun the
            # GEMM (closes the WAR race), at zero expected stall.
            gemm_half(0, "hi", first=True)
            gemm_half(0, "lo", last=True)
            ow0, oi0 = finish_group(0)
            emit_out(0, ow0, oi0)  # paces: lo2/hi3 reuse g0-read buffers
            xbar("hi", 2)
            xbar("lo", 2)
            gemm_half(1, "hi", first=True)
            gemm_half(1, "lo", last=True)
            ow1, oi1 = finish_group(1)
            emit_out(1, ow1, oi1)  # paces: lo3 reuses g1-read buffers
            xbar("hi", 3)
            xbar("lo", 3, 0, NCH // 2)
            xbar("lo", 3, NCH // 2, NCH)
            gemm_half(2, "hi", first=True)
            gemm_half(2, "lo", last=True)
            ow2, oi2 = finish_group(2)
            emit_out(2, ow2, oi2)
            gemm_half(3, "hi", first=True)
            gemm_half(3, "lo", 0, NCH // 2)
            gemm_half(3, "lo", NCH // 2, NCH, last=True)
            ow3, oi3 = finish_group(3)
            emit_out(3, ow3, oi3)

    return nc


_NC_CACHE = {}


def _get_nc(tok: int):
    if tok not in _NC_CACHE:
        _NC_CACHE[tok] = build_nc(tok)
    return _NC_CACHE[tok]


def _split_hi_lo(a: np.ndarray):
    import ml_dtypes
    hi = a.astype(ml_dtypes.bfloat16)
    lo = (a - hi.astype(np.float32)).astype(ml_dtypes.bfloat16)
    return hi, lo


def make_in_maps(x: np.ndarray, gate_w: np.ndarray):
    """Shard + reformat full inputs into per-core input maps."""
    import ml_dtypes
    xf = np.ascontiguousarray(x.reshape(TOK_TOTAL, H), dtype=np.float32)
    hi_f, lo_f = _split_hi_lo(xf)

    w_hi, w_lo = _split_hi_lo(np.asarray(gate_w, dtype=np.float32))

    def chunked(w):  # wt[p, c, e] = w[e, 128c + p]
        return np.ascontiguousarray(
            w.T.reshape(NCH, P, E).transpose(1, 0, 2))

    wa = np.ascontiguousarray(
        np.concatenate([chunked(w_hi), chunked(w_lo)], axis=2))
    sumid = np.concatenate([np.eye(E), np.eye(E)], axis=0).astype(np.float32)

    maps = []
    for i in range(N_CORES):
        sl = slice(i * TOK, (i + 1) * TOK)
        maps.append({
            "xhi": np.ascontiguousarray(hi_f[sl]),
            "xlo": np.ascontiguousarray(lo_f[sl]),
            "wa": wa, "sumid": sumid,
        })
    return maps


def kernel(x, gate_w, _trace: bool = False):
    x = np.asarray(x, dtype=np.float32)
    gate_w = np.asarray(gate_w, dtype=np.float32)
    nc = _get_nc(TOK)
    in_maps = make_in_maps(x, gate_w)
    res = run_bass_kernel_spmd(
        nc, in_maps, core_ids=list(range(N_CORES)), trace=_trace
    )
    out_w = np.concatenate([res.results[i]["out_w"] for i in range(N_CORES)])
    out_i = np.concatenate([res.results[i]["out_i"] for i in range(N_CORES)])
    topk_weights = out_w.reshape(B, S, 2)
    topk_indices = out_i.astype(np.int32).reshape(B, S, 2)
    if _trace:
        kernel._last_result = res
    return topk_weights, topk_indices


# revision 3
# speedup vs baseline: 1.0360x; 1.0360x over previous
"""MoE router kernel for Trainium2 (8 NeuronCores, SPMD data-parallel), v3.

Computes, for x [B,S,H] and gate_w [E,H]:
    logits = x @ gate_w.T; p = softmax(logits); top-2 renormalized
    => w1 = sigmoid(l1 - l2), w2 = sigmoid(l2 - l1).

v3 = hybrid input engine:
- Host splits x into bf16 hi/lo (same total bytes as fp32) and gate_w
  into stacked bf16 blocks Wa = [w_hi | w_lo], Wb = [w_lo | w_hi].
- Groups 0-2 (6 slabs) load via DMA xbar transpose (HBM -> SBUF already
  transposed), all serialized on the sync queue (the xbar is a single
  shared unit; concurrent xbar transposes corrupt; plain DMA alongside
  is safe - probed).
- Group 3 loads via plain DMA (scalar queue) + PE bf16 transposes,
  statically interleaved into PE idle slots so the PE stays warm.
- GEMM per h-chunk c accumulates into PSUM bank A [128, 512]:
    MM_a: lhsT = [w_hi_c | w_lo_c], rhs = hiT_c
    MM_b: lhsT = [w_lo_c | w_hi_c], rhs = loT_c
  logits = A[0:64] + A[64:128] = (hi+lo) @ (w_hi+w_lo)  (~17-bit).
  The halves-add is folded into the transpose-back matmul vs [I64; I64].
- Output DMAs are deferred to the end (they must not block the xbar
  stream mid-kernel).
"""

import sys

sys.path.insert(0, "/opt/trn_rl_repo")

import numpy as np

import concourse.bass as bass
import concourse.mybir as mybir
import concourse.tile as tile
from concourse.bass_utils import run_bass_kernel_spmd
import orjson
import concourse.bass_utils as _bu
import concourse.bass2jax as _b2j

_orig_compile_bir = _bu.compile_bir_kernel


def _legalize_waits(bir_json: bytes) -> bytes:
    """This walrus build allows only ONE sync-wait per compute
    instruction; move excess waits onto a Drain inserted just before."""
    m = orjson.loads(bir_json)
    changed = False
    for fn in m["functions"]:
        for blk in fn["blocks"]:
            out = []
            for inst in blk["instructions"]:
                si = inst.get("sync_info")
                w = (si or {}).get("on_wait") or []
                if len(w) > 1:
                    for k, wk in enumerate(w[:-1]):
                        out.append({
                            "debug": inst.get("debug", 0),
                            "engine": inst["engine"],
                            "ins": [], "outs": [],
                            "name": inst["name"] + f"-lw{k}",
                            "opcode": "Drain",
                            "sync_info": {"on_update": [], "on_wait": [wk]},
                        })
                    si["on_wait"] = w[-1:]
                    changed = True
                out.append(inst)
            blk["instructions"] = out
    return orjson.dumps(m) if changed else bir_json


def _compile_bir_legalized(bir_json, tmpdir, neff_name="file.neff"):
    return _orig_compile_bir(_legalize_waits(bir_json), tmpdir, neff_name)


_bu.compile_bir_kernel = _compile_bir_legalized
_b2j.compile_bir_kernel = _compile_bir_legalized

F32 = mybir.dt.float32
U32 = mybir.dt.uint32
BF16 = mybir.dt.bfloat16

B, S, H, E = 4, 4096, 4096, 64
N_CORES = 8
P = 128
TOK_TOTAL = B * S            # 16384
TOK = TOK_TOTAL // N_CORES   # 2048 tokens per core
NCH = H // P                 # 32 contraction chunks of 128
GTOK = 512                   # tokens per group (one PSUM bank of fp32)
NG = TOK // GTOK             # 4 groups
PE_GROUP = 3                 # this group's slabs go via plain DMA + PE


def build_nc(tok: int = TOK):
    nc = bass.Bass()

    xhi_ext = nc.declare_dram_parameter("xhi", [tok, H], BF16, isOutput=False)
    xlo_ext = nc.declare_dram_parameter("xlo", [tok, H], BF16, isOutput=False)
    wa_ext = nc.declare_dram_parameter("wa", [P, NCH, P], BF16, isOutput=False)
    sid_ext = nc.declare_dram_parameter("sumid", [P, E], F32, isOutput=False)
    pace_ext = nc.declare_dram_parameter("pace", [NG, 4], F32, isOutput=True)
    ow_ext = nc.declare_dram_parameter("out_w", [tok, 2], F32, isOutput=True)
    oi_ext = nc.declare_dram_parameter("out_i", [tok, 2], U32, isOutput=True)

    with tile.TileContext(nc) as tc:
        with (
            tc.tile_pool(name="consts", bufs=1) as consts,
            tc.tile_pool(name="xb", bufs=5) as xbp,        # xbar-landed slabs
            tc.tile_pool(name="ps_a", bufs=2, space="PSUM") as psA,
            tc.tile_pool(name="ps_t", bufs=2, space="PSUM") as psT,
            tc.tile_pool(name="lgp", bufs=2) as lgp,
            tc.tile_pool(name="small", bufs=4) as small,
            tc.tile_pool(name="outp", bufs=2 * NG) as outp,
        ):
            # ---- consts (scalar queue; sync queue is reserved for xbar) ----
            wa_sb = consts.tile([P, NCH, P], BF16)
            nc.scalar.dma_start(wa_sb[:], wa_ext[:])
            sid_sb = consts.tile([P, E], F32)
            nc.scalar.dma_start(sid_sb[:], sid_ext[:])

            # Engine primers.
            prim = consts.tile([P, 2], F32)
            nc.vector.memset(prim[:, 0:1], 0.0)
            nc.scalar.copy(prim[:, 1:2], nc.const_aps.tensor(1.0, (P, 1)))
            with tc.tile_pool(name="scr", bufs=1, space="PSUM") as scr_pool:
                scr = scr_pool.tile([P, P], F32)
                nc.tensor.matmul(scr[0:E, 0:E], sid_sb[:], sid_sb[:],
                                 start=True, stop=True)
                nc.tensor.matmul(scr[:], wa_sb[:, 0, :], wa_sb[:, 0, :],
                                 start=True, stop=True)

            # ---- xbar transposes for groups 0-2 (sync queue, serial) ----
            slabs = {}
            for half, g in (("hi", 0), ("lo", 0), ("hi", 1), ("lo", 1),
                            ("hi", 2), ("lo", 2), ("hi", 3), ("lo", 3)):
                slabs[(half, g)] = xbp.tile([P, NCH, GTOK], BF16,
                                            name="xslab")

            def xbar(half, g, c0=0, c1=NCH):
                t0 = g * GTOK
                ext = xhi_ext if half == "hi" else xlo_ext
                nc.sync.dma_start(
                    slabs[(half, g)][:, c0:c1],
                    ext[t0:t0 + GTOK, c0 * P:c1 * P],
                    transpose=True)

            xbar("hi", 0)
            xbar("lo", 0)
            xbar("hi", 1)
            xbar("lo", 1)

            # ---- PE-side helpers ----
            A_tiles = {}

            def gemm_half(g, half, c0=0, c1=NCH, first=False, last=False):
                A = A_tiles[g]
                w_sb = wa_sb
                src = slabs[(half, g)]
                for c in range(c0, c1):
                    nc.tensor.matmul(A[:], w_sb[:, c, :], src[:, c, :],
                                     start=(first and c == c0),
                                     stop=(last and c == c1 - 1))

            def finish_group(g):
                """As copy (DVE), sum-matmuls, lgt copy, topk, sigmoid."""
                A = A_tiles[g]
                As = lgp.tile([P, GTOK], F32)
                nc.vector.tensor_copy(As[:], A[:])
                ltb = psT.tile([P, 4, E], F32)
                for t in range(4):
                    nc.tensor.matmul(ltb[:, t, :],
                                     As[:, t * P:(t + 1) * P], sid_sb[:],
                                     start=True, stop=True)
                ow_t = outp.tile([P, 4, 2], F32)
                oi_t = outp.tile([P, 4, 2], U32)
                for t in range(4):
                    mx = small.tile([P, 8], F32)
                    nc.vector.max(mx[:], ltb[:, t, :])
                    ix = small.tile([P, 8], U32)
                    nc.vector.max_index(ix[:], mx[:], ltb[:, t, :])
                    nc.scalar.activation(
                        ow_t[:, t, 0:1], mx[:, 1:2],
                        mybir.ActivationFunctionType.Sigmoid,
                        bias=mx[:, 0:1], scale=-1.0)
                    nc.scalar.activation(
                        ow_t[:, t, 1:2], mx[:, 0:1],
                        mybir.ActivationFunctionType.Sigmoid,
                        bias=mx[:, 1:2], scale=-1.0)
                    nc.vector.tensor_copy(oi_t[:, t, :], ix[:, 0:2])
                return ow_t, oi_t, As

            # ---- static PE schedule (interleave transposes into idle) ----
            for g in (0, 3, 1, 2):
                A_tiles[g] = psA.tile([P, GTOK], F32, name="A")

            def emit_out(g, ow_t, oi_t):
                t0 = g * GTOK
                nc.sync.dma_start(
                    ow_ext[t0:t0 + GTOK, :].rearrange("(t p) k -> p t k", p=P),
                    ow_t[:])
                nc.sync.dma_start(
                    oi_ext[t0:t0 + GTOK, :].rearrange("(t p) k -> p t k", p=P),
                    oi_t[:])

            # Pacer: a 16-byte DMA reading the group's As tile. It waits
            # (transitively) on every GEMM matmul of the group, so the slab
            # buffers g's GEMM read can be safely overwritten by any xbar
            # transpose queued after it -- without waiting for the slower
            # top-k/sigmoid chain like a real output DMA would.
            gemm_half(0, "hi", first=True)
            gemm_half(0, "lo", last=True)
            ow0, oi0, As0 = finish_group(0)
            nc.sync.dma_start(pace_ext[0:1, :], As0[0:1, 0:4])
            xbar("hi", 2)
            xbar("lo", 2)
            gemm_half(1, "hi", first=True)
            gemm_half(1, "lo", last=True)
            ow1, oi1, As1 = finish_group(1)
            nc.sync.dma_start(pace_ext[1:2, :], As1[0:1, 0:4])
            xbar("hi", 3)
            xbar("lo", 3, 0, NCH // 2)
            xbar("lo", 3, NCH // 2, NCH)
            gemm_half(2, "hi", first=True)
            gemm_half(2, "lo", last=True)
            ow2, oi2, _ = finish_group(2)
            gemm_half(3, "hi", first=True)
            gemm_half(3, "lo", 0, NCH // 2)
            gemm_half(3, "lo", NCH // 2, NCH, last=True)
            ow3, oi3, _ = finish_group(3)
            emit_out(0, ow0, oi0)
            emit_out(1, ow1, oi1)
            emit_out(2, ow2, oi2)
            emit_out(3, ow3, oi3)

    return nc


_NC_CACHE = {}


def _get_nc(tok: int):
    if tok not in _NC_CACHE:
        _NC_CACHE[tok] = build_nc(tok)
    return _NC_CACHE[tok]


def _split_hi_lo(a: np.ndarray):
    import ml_dtypes
    hi = a.astype(ml_dtypes.bfloat16)
    lo = (a - hi.astype(np.float32)).astype(ml_dtypes.bfloat16)
    return hi, lo


def make_in_maps(x: np.ndarray, gate_w: np.ndarray):
    """Shard + reformat full inputs into per-core input maps."""
    import ml_dtypes
    xf = np.ascontiguousarray(x.reshape(TOK_TOTAL, H), dtype=np.float32)
    hi_f, lo_f = _split_hi_lo(xf)

    w_hi, w_lo = _split_hi_lo(np.asarray(gate_w, dtype=np.float32))

    def chunked(w):  # wt[p, c, e] = w[e, 128c + p]
        return np.ascontiguousarray(
            w.T.reshape(NCH, P, E).transpose(1, 0, 2))

    wa = np.ascontiguousarray(
        np.concatenate([chunked(w_hi), chunked(w_lo)], axis=2))
    sumid = np.concatenate([np.eye(E), np.eye(E)], axis=0).astype(np.float32)

    maps = []
    for i in range(N_CORES):
        sl = slice(i * TOK, (i + 1) * TOK)
        maps.append({
            "xhi": np.ascontiguousarray(hi_f[sl]),
            "xlo": np.ascontiguousarray(lo_f[sl]),
            "wa": wa, "sumid": sumid,
        })
    return maps


def kernel(x, gate_w, _trace: bool = False):
    x = np.asarray(x, dtype=np.float32)
    gate_w = np.asarray(gate_w, dtype=np.float32)
    nc = _get_nc(TOK)
    in_maps = make_in_maps(x, gate_w)
    res = run_bass_kernel_spmd(
        nc, in_maps, core_ids=list(range(N_CORES)), trace=_trace
    )
    out_w = np.concatenate([res.results[i]["out_w"] for i in range(N_CORES)])
    out_i = np.concatenate([res.results[i]["out_i"] for i in range(N_CORES)])
    topk_weights = out_w.reshape(B, S, 2)
    topk_indices = out_i.astype(np.int32).reshape(B, S, 2)
    if _trace:
        kernel._last_result = res
    return topk_weights, topk_indices
